# revision 1
# baseline (speedup 1.0000x reference)
"""CT parallel-beam 2D forward projector on 8 Trainium2 NeuronCores.

Algorithm (exact, validated vs reference to ~1.6e-5 rel err):
  For each view angle, the trapezoid-footprint bin weights are written via the
  trapezoid CDF  Phic(t) = q*[relu^2(t) - relu^2(t-B) - relu^2(t-A) + relu^2(t-A-B)]
  (A = max(|cos|,sin), B = min, q = 1/(2AB)).  With the separable floor split
  z = p_xi(xi) + p_eta(eta),  b_xi = floor(p_xi), b_eta = floor(p_eta),
  g = frac_xi + frac_eta in [0,2), every pixel scatters into bins
  n = b_xi + b_eta + j (j = 0..3) with weights U_j(g) = Phi_{j+1}(g) - Phi_j(g),
  Phi_i(g) = Phic(i - 1/2 - g), Phi_0 = 0, Phi_4 = 1 (the floor carry is absorbed
  by the continuous U_j).  Abel summation turns the 4 tap fields into gathers of
  T_i = img * Phi_i (i = 1..3) and img itself at slots i-1 (+) and i (-).

  Device pipeline per (angle, 128-row eta-chunk), layout [eta-part, xi-free]:
    ACT : y_t = Relu(-f_xi + (E_t - f_eta))  for 12 constants E_t, r_t = y_t^2
    DVE : Phi_i combine, T_i = (s*q)*img, plus run-sum S halves
    GPSIMD: indirect_copy gathers (monotone xi->bin binning, host-built indices)
    PE  : one-hot matmul over eta (local per-chunk bins v' < 96), PSUM-accumulated
          over the 7 signed gather instances
  Host: tiny anti-diagonal collapse R[v',m] -> proj[n], plus direct numpy path for
  the two degenerate axis-aligned angles (B ~ 0).

SPMD: one program for all 8 cores. Cores 0-3 process "class X" angles
(|cos| >= sin) on img; cores 4-7 process "class Y" angles on img.T. All
per-angle variation (tables, one-hots, gather indices) is input data.
"""

import numpy as np

Nx = Ny = 512
Nu = 768
NTHETA = 180
HALF_U = (Nu - 1) / 2.0
NCORES = 8
import os as _os
APC = int(_os.environ.get("CT_APC", "23"))   # angles per core
NCHUNK = 4        # eta chunks of 128
MPAD = 528        # gather output width (W <= 513, padded, mult of 16)
RPAD = 544        # R output width (W + 3 <= 516, plus pad)
PS1W = 32         # second PSUM piece width (covers m in [512, 531))
VP = 96           # local v' bins per chunk (128*0.7072 < 91)
ZERO_COL = 1023   # index of the all-zero column in each C buffer
B_RECT = 1e-4     # below this min-slope, use the host rect path

_PROGRAM_CACHE = {}


# --------------------------------------------------------------------------
# host tables
# --------------------------------------------------------------------------

def _angle_tables(theta_val):
    th = float(theta_val)
    c, s = np.cos(th), np.sin(th)
    ac, asn = abs(c), abs(s)
    A, B = max(ac, asn), min(ac, asn)
    b2 = ac + asn
    cls = 0 if ac >= asn else 1
    a_xi, a_eta = (c, s) if cls == 0 else (s, c)
    z0 = HALF_U - b2 / 2 - 255.5 * (c + s)
    grid = np.arange(512)
    pxi = a_xi * grid + z0
    peta = a_eta * grid
    bxi = np.floor(pxi).astype(np.int64)
    fxi = pxi - bxi
    beta = np.floor(peta).astype(np.int64)
    feta = peta - beta
    q = 1.0 / (2 * A * B) if B > B_RECT else None
    return dict(c=c, s=s, A=A, B=B, b2=b2, q=q, cls=cls,
                bxi=bxi, fxi=fxi, beta=beta, feta=feta)


def _gather_tables(T):
    """xi-binning run-starts and the 7 instance index streams (length MPAD)."""
    bxi = T["bxi"]
    bxi_min = int(bxi.min())
    mloc = bxi - bxi_min
    W = int(mloc.max()) + 1
    # run start xa[m] and length L[m] (1 or 2) for each bin m
    xa = np.zeros(W, dtype=np.int64)
    L = np.zeros(W, dtype=np.int64)
    order = np.argsort(mloc, kind="stable")
    sorted_m = mloc[order]
    first = np.searchsorted(sorted_m, np.arange(W), side="left")
    last = np.searchsorted(sorted_m, np.arange(W), side="right")
    for m in range(W):
        idxs = order[first[m]:last[m]]
        n = len(idxs)
        assert 1 <= n <= 2
        xa[m] = idxs.min()
        L[m] = n
        if n == 2:
            assert idxs.max() - idxs.min() == 1

    # single zero-shift stream; slot shifts are applied as PSUM column offsets
    idx = np.full(MPAD, ZERO_COL, dtype=np.int64)
    msrc = np.arange(0, min(W, MPAD))
    idx[:len(msrc)] = np.where(L[msrc] == 2, 512 + xa[msrc], xa[msrc])
    return dict(bxi_min=bxi_min, W=W, stream=idx)


def _wrap_idx(stream):
    """[MPAD] int -> [128, MPAD//16] uint16 wrapped per 16-partition groups."""
    w = stream.reshape(MPAD // 16, 16).T.astype(np.uint16)   # [16, MPAD/16]
    return np.tile(w, (8, 1))                                 # [128, MPAD/16]


def _core_inputs(img_layout, angle_list, tables):
    """Build the input map for one core. img_layout: [512,512] f32 in [eta,xi]."""
    A_ = APC
    fxi_t = np.zeros((A_, 512), dtype=np.float32)
    bias_t = np.zeros((A_, NCHUNK, 128, 16), dtype=np.float32)
    oh_t = np.zeros((A_, NCHUNK, 128, VP), dtype=np.float32)
    idx_t = np.zeros((A_, 128, MPAD // 16), dtype=np.uint16)
    meta = []
    for ai, a in enumerate(angle_list):
        T = tables[a]
        G = _gather_tables(T)
        fxi_t[ai] = T["fxi"].astype(np.float32)
        knots = [0.0, T["B"], T["A"], T["A"] + T["B"]]
        feta = T["feta"]
        beta = T["beta"]
        for k in range(NCHUNK):
            sl = slice(k * 128, (k + 1) * 128)
            col = 0
            for i in (1, 2, 3):
                for kn in knots:
                    E = i - 0.5 - kn
                    bias_t[ai, k, :, col] = (E - feta[sl]).astype(np.float32)
                    col += 1
            bias_t[ai, k, :, 12] = np.float32(T["q"])
            bias_t[ai, k, :, 13] = feta[sl].astype(np.float32)
            vloc = beta[sl] - beta[sl].min()
            assert vloc.min() >= 0 and vloc.max() < VP, (vloc.min(), vloc.max())
            oh_t[ai, k, np.arange(128), vloc] = 1.0
        idx_t[ai] = _wrap_idx(G["stream"])
        meta.append(dict(angle=a, bxi_min=G["bxi_min"], W=G["W"],
                         beta0=[int(beta[k * 128:(k + 1) * 128].min())
                                for k in range(NCHUNK)]))
    in_map = {
        "imgL": np.ascontiguousarray(img_layout).astype(np.float32),
        "fxi_t": fxi_t,
        "bias_t": bias_t,
        "oh_t": oh_t,
        "idx_t": idx_t,
    }
    return in_map, meta


# --------------------------------------------------------------------------
# the bass program (identical for all cores)
# --------------------------------------------------------------------------

def _build_program():
    if "nc" in _PROGRAM_CACHE:
        return _PROGRAM_CACHE["nc"], _PROGRAM_CACHE["io"]

    import concourse.bass as bass
    import concourse.tile as tile
    from concourse import bacc, mybir
    from contextlib import ExitStack

    dt = mybir.dt
    AF = mybir.ActivationFunctionType
    ALU = mybir.AluOpType

    # engine assignment config. A=ACT, D=DVE, G=GPSIMD.
    # iform: per-i pipeline form (A = ACT relu+square, D = DVE min+products)
    cfg_s = _os.environ.get(
        "CT_CFG",
        "iform=AAD;sq=AAAAAAAAAAAA;comb=DDDDDDDDD;dcomb=DDDDDDD;ts=D;"
        "shalf=DDDD;imgcopy=A;drain=A")  # best of TimelineSim sweep (1.25 ms)
    CFG = dict(kv.split("=") for kv in cfg_s.split(";"))
    _PROGRAM_CACHE["cfg"] = CFG

    nc = bacc.Bacc("TRN2", target_bir_lowering=False, debug=False,
                   num_devices=NCORES)

    imgL = nc.dram_tensor("imgL", [512, 512], dt.float32, kind="ExternalInput").ap()
    fxi_t = nc.dram_tensor("fxi_t", [APC, 512], dt.float32, kind="ExternalInput").ap()
    bias_t = nc.dram_tensor("bias_t", [APC, NCHUNK, 128, 16], dt.float32,
                            kind="ExternalInput").ap()
    oh_t = nc.dram_tensor("oh_t", [APC, NCHUNK, 128, VP], dt.float32,
                          kind="ExternalInput").ap()
    idx_t = nc.dram_tensor("idx_t", [APC, 128, MPAD // 16], dt.uint16,
                           kind="ExternalInput").ap()
    r_out = nc.dram_tensor("r_out", [APC, NCHUNK, VP, RPAD], dt.float32,
                           kind="ExternalOutput").ap()

    # (field, psum column shift, sign); order chosen so the first writer of
    # each PSUM tile covers its full written range (start=True coverage)
    instances = [(0, 0, +1), (3, 3, +1), (1, 1, +1), (2, 2, +1),
                 (0, 1, -1), (1, 2, -1), (2, 3, -1)]

    with tile.TileContext(nc) as tc, ExitStack() as ctx:
        BB = int(_os.environ.get("CT_BUFS", "0"))  # 1 = bigger pools
        img_pool = ctx.enter_context(tc.tile_pool(name="img", bufs=1))
        row_pool = ctx.enter_context(tc.tile_pool(name="rows", bufs=2))
        tab_pool = ctx.enter_context(tc.tile_pool(name="tabs", bufs=2 + BB))
        y_pool = ctx.enter_context(tc.tile_pool(name="ys", bufs=3 + BB))
        r_pool = ctx.enter_context(tc.tile_pool(name="rs", bufs=3 + BB))
        ph_pool = ctx.enter_context(tc.tile_pool(name="phi", bufs=2 + BB))
        c_pool = ctx.enter_context(tc.tile_pool(name="cbuf", bufs=2 + BB))
        g_pool = ctx.enter_context(tc.tile_pool(name="gath", bufs=2 + BB))
        ps_pool = ctx.enter_context(tc.tile_pool(name="psum", bufs=2, space="PSUM"))
        o_pool = ctx.enter_context(tc.tile_pool(name="outs", bufs=2 + BB))

        # resident image chunks
        img_ch = []
        for k in range(NCHUNK):
            t = img_pool.tile([128, 512], dt.float32, tag=f"imgc{k}")
            nc.sync.dma_start(t[:], imgL[k * 128:(k + 1) * 128, :])
            img_ch.append(t)

        for ai in range(APC):
            fxi_bt = row_pool.tile([128, 512], dt.float32, tag="fxib")
            nc.sync.dma_start(fxi_bt[:],
                              fxi_t[ai:ai + 1, :].to_broadcast([128, 512]))
            fxi_b = fxi_bt[:]

            idxt = tab_pool.tile([128, MPAD // 16], dt.uint16, tag="idx")
            nc.sync.dma_start(idxt[:], idx_t[ai])

            for k in range(NCHUNK):
                bias = tab_pool.tile([128, 16], dt.float32, tag="bias")
                nc.sync.dma_start(bias[:], bias_t[ai, k])
                oh = tab_pool.tile([128, VP], dt.float32, tag="oh")
                nc.sync.dma_start(oh[:], oh_t[ai, k])
                ohn = tab_pool.tile([128, VP], dt.float32, tag="ohn")
                nc.vector.tensor_scalar(ohn[:], oh[:], -1.0, None, ALU.mult)

                qAP = bias[:, 12:13]

                # C buffers: [F(512) | S(511) | pad | zero col]
                cbufs = []
                for f in range(4):
                    cb = c_pool.tile([128, 1024], dt.float32, tag=f"c{f}")
                    cbufs.append(cb)

                # engine helpers for load balancing (cfg chars: A/D/G)
                def eng(ch):
                    return {"A": nc.scalar, "D": nc.vector, "G": nc.gpsimd}[ch]

                def tt(ch, out, a, b, op):
                    if ch == "A":
                        ch = "D"  # ACT has no tensor_tensor
                    eng(ch).tensor_tensor(out, a, b, op)

                # img field straight into C3
                if CFG["imgcopy"] == "A":
                    nc.scalar.copy(cbufs[3][:, 0:512], img_ch[k][:])
                else:
                    eng(CFG["imgcopy"]).tensor_copy(cbufs[3][:, 0:512], img_ch[k][:])

                # --- 12 relu^2 terms + Phi combine (two alternative forms)
                for i in range(3):
                    form = CFG["iform"][i]
                    if form in ("A", "H"):
                        ys = []
                        for kn in range(4):
                            y = y_pool.tile([128, 512], dt.float32, tag=f"y{kn}")
                            if form == "A":
                                nc.scalar.activation(
                                    y[:], fxi_b, AF.Relu,
                                    bias=bias[:, 4 * i + kn:4 * i + kn + 1],
                                    scale=-1.0)
                            else:
                                # y' = min(g-E, 0) = -relu(E-g); y'^2 == relu^2
                                eng(CFG["ts"]).tensor_scalar(
                                    y[:], fxi_b,
                                    bias[:, 4 * i + kn:4 * i + kn + 1],
                                    0.0, ALU.subtract, ALU.min)
                            ys.append(y)
                        rs = []
                        for kn in range(4):
                            t = 4 * i + kn
                            r = r_pool.tile([128, 512], dt.float32, tag=f"r{kn}")
                            ch = CFG["sq"][t]
                            if ch == "A":
                                nc.scalar.activation(r[:], ys[kn][:], AF.Square)
                            else:
                                eng(ch).tensor_tensor(r[:], ys[kn][:], ys[kn][:],
                                                      ALU.mult)
                            rs.append(r)
                        s12 = ph_pool.tile([128, 512], dt.float32, tag="s12")
                        tt(CFG["comb"][3 * i + 0], s12[:], rs[0][:], rs[1][:],
                           ALU.subtract)
                        s34 = ph_pool.tile([128, 512], dt.float32, tag="s34")
                        tt(CFG["comb"][3 * i + 1], s34[:], rs[3][:], rs[2][:],
                           ALU.subtract)
                        ssum = ph_pool.tile([128, 512], dt.float32, tag="ssum")
                        tt(CFG["comb"][3 * i + 2], ssum[:], s12[:], s34[:],
                           ALU.add)
                    else:
                        # y'_kn = min(g - E, 0) = -relu(E - g); r = y'^2
                        # r1-r2 = (y1'-y2')(y1'+y2'), r4-r3 = (y4'-y3')(y4'+y3')
                        ys = []
                        for kn in range(4):
                            y = y_pool.tile([128, 512], dt.float32, tag=f"y{kn}")
                            eng(CFG["ts"]).tensor_scalar(
                                y[:], fxi_b, bias[:, 4 * i + kn:4 * i + kn + 1],
                                0.0, ALU.subtract, ALU.min)
                            ys.append(y)
                        d12 = ph_pool.tile([128, 512], dt.float32, tag="s12")
                        tt(CFG["dcomb"][0], d12[:], ys[0][:], ys[1][:],
                           ALU.subtract)
                        a12 = r_pool.tile([128, 512], dt.float32, tag="r0")
                        tt(CFG["dcomb"][1], a12[:], ys[0][:], ys[1][:], ALU.add)
                        m12 = r_pool.tile([128, 512], dt.float32, tag="r1")
                        tt(CFG["dcomb"][2], m12[:], d12[:], a12[:], ALU.mult)
                        d34 = ph_pool.tile([128, 512], dt.float32, tag="s34")
                        tt(CFG["dcomb"][3], d34[:], ys[3][:], ys[2][:],
                           ALU.subtract)
                        a34 = r_pool.tile([128, 512], dt.float32, tag="r2")
                        tt(CFG["dcomb"][4], a34[:], ys[3][:], ys[2][:], ALU.add)
                        m34 = r_pool.tile([128, 512], dt.float32, tag="r3")
                        tt(CFG["dcomb"][5], m34[:], d34[:], a34[:], ALU.mult)
                        ssum = ph_pool.tile([128, 512], dt.float32, tag="ssum")
                        tt(CFG["dcomb"][6], ssum[:], m12[:], m34[:], ALU.add)
                    # T_i = (ssum * q) * img  -> C_i F-half
                    nc.vector.scalar_tensor_tensor(
                        cbufs[i][:, 0:512], ssum[:], qAP, img_ch[k][:],
                        ALU.mult, ALU.mult)

                # S halves: C[:, 512:1023] = F[:, 0:511] + F[:, 1:512]
                for f in range(4):
                    tt(CFG["shalf"][f], cbufs[f][:, 512:1023],
                       cbufs[f][:, 0:511], cbufs[f][:, 1:512], ALU.add)
                    nc.vector.memset(cbufs[f][:, 1023:1024], 0.0)

                # --- 4 gathers (T1, T2, T3, IMG), one shared index stream
                gts = []
                for f in range(4):
                    gt = g_pool.tile([128, MPAD], dt.float32, tag=f"g{f}")
                    nc.gpsimd.indirect_copy(gt[:], cbufs[f][:], idxt[:], True)
                    gts.append(gt)

                # --- PE one-hot eta-binning; slot shifts via PSUM col offsets
                # ps0 covers output m in [0, 512); ps1 covers [512, 531)
                ps0 = ps_pool.tile([VP, 512], dt.float32, tag="ps0")
                ps1 = ps_pool.tile([VP, PS1W], dt.float32, tag="ps1")
                mms = []  # (tile_id, out_ap, lhs, rhs_ap)
                for f, s, sgn in instances:
                    lhs = oh if sgn > 0 else ohn
                    mms.append((0, ps0[:, s:512], lhs, gts[f][:, 0:512 - s]))
                    mms.append((1, ps1[:, 0:16 + s], lhs,
                                gts[f][:, 512 - s:528]))
                # order: first full-coverage writer per tile, then the rest
                order = [0, 3, 1, 2] + list(range(4, 14))
                started = {0: False, 1: False}
                for pos, mi in enumerate(order):
                    tid, out_ap, lhs, rhs_ap = mms[mi]
                    is_first = not started[tid]
                    started[tid] = True
                    is_last = (pos == max(p for p, m in enumerate(order)
                                          if mms[m][0] == tid))
                    nc.tensor.matmul(out_ap, lhs[:], rhs_ap,
                                     start=is_first, stop=is_last)

                rout = o_pool.tile([VP, RPAD], dt.float32, tag="rout")
                if CFG["drain"] == "A":
                    nc.scalar.copy(rout[:, 0:512], ps0[:])
                    nc.scalar.copy(rout[:, 512:531], ps1[:, 0:19])
                else:
                    nc.vector.tensor_copy(rout[:, 0:512], ps0[:])
                    nc.vector.tensor_copy(rout[:, 512:531], ps1[:, 0:19])
                nc.vector.memset(rout[:, 531:RPAD], 0.0)
                nc.sync.dma_start(r_out[ai, k][:, 0:RPAD], rout[:])

    nc.compile()
    _PROGRAM_CACHE["nc"] = nc
    _PROGRAM_CACHE["io"] = None
    return nc, None


# --------------------------------------------------------------------------
# host-side rect path (degenerate angles) — numpy port of the reference
# --------------------------------------------------------------------------

def _host_project(img, theta_vals):
    y = (np.arange(Ny) - (Ny - 1) / 2.0)
    x = (np.arange(Nx) - (Nx - 1) / 2.0)
    y2d, x2d = np.meshgrid(y, x, indexing="ij")
    img_v = img.reshape(-1).astype(np.float64)
    out = np.zeros((len(theta_vals), Nu), dtype=np.float64)
    K = 4
    for t, th in enumerate(theta_vals):
        th = float(th)
        cos_t, sin_t = np.cos(th), np.sin(th)
        ac, asn = abs(cos_t), abs(sin_t)
        h = min(1.0 / ac if ac > 0 else np.inf, 1.0 / asn if asn > 0 else np.inf)
        b1 = abs(asn - ac)
        b2 = abs(asn + ac)
        u0 = x2d * cos_t + y2d * sin_t
        u1 = u0 - b2 / 2
        u2 = u0 - b1 / 2
        u3 = u0 + b1 / 2
        u4 = u0 + b2 / 2
        base = np.floor(u1 + HALF_U).astype(np.int64)
        den12 = (u2 - u1) + (u1 == u2)
        den34 = (u4 - u3) + (u3 == u4)
        acc = np.zeros(Nu + 8, dtype=np.float64)
        for k in range(K):
            idx = base + k
            u = idx - HALF_U
            lo, hi = u - 0.5, u + 0.5
            uA = np.maximum(u1, lo); uB = np.minimum(u2, hi)
            w = (uB > uA) * (h / (2.0 * den12)) * ((uB - u1) ** 2 - (uA - u1) ** 2)
            uA = np.maximum(u2, lo); uB = np.minimum(u3, hi)
            w = w + (uB > uA) * h * (uB - uA)
            uA = np.maximum(u3, lo); uB = np.minimum(u4, hi)
            w = w + (uB > uA) * (h / (2.0 * den34)) * ((uA - u4) ** 2 - (uB - u4) ** 2)
            np.add.at(acc, np.clip(idx.reshape(-1), 0, Nu - 1),
                      img_v * w.reshape(-1))
        out[t] = acc[:Nu]
    return out.astype(np.float32)


# --------------------------------------------------------------------------
# main entry
# --------------------------------------------------------------------------

def kernel(img, theta):
    img = np.asarray(img, dtype=np.float32)
    theta = np.asarray(theta, dtype=np.float32)
    assert img.shape == (Ny, Nx) and theta.shape == (NTHETA,)

    tables = {a: _angle_tables(theta[a]) for a in range(NTHETA)}
    rect_angles = [a for a in range(NTHETA) if tables[a]["q"] is None]
    dev_angles = [a for a in range(NTHETA) if tables[a]["q"] is not None]
    clsX = [a for a in dev_angles if tables[a]["cls"] == 0]
    clsY = [a for a in dev_angles if tables[a]["cls"] == 1]
    assert len(clsX) <= 4 * APC and len(clsY) <= 4 * APC

    # interleave class angles over 4 cores each, pad with repeats
    def assign(lst, ncores):
        groups = [lst[i::ncores] for i in range(ncores)]
        return [g + [g[-1]] * (APC - len(g)) if g else [dev_angles[0]] * APC
                for g in groups]

    coreX = assign(clsX, 4)
    coreY = assign(clsY, 4)
    core_angles = coreX + coreY

    imgT = np.ascontiguousarray(img.T)
    in_maps, metas = [], []
    for ci in range(NCORES):
        layout = img if ci < 4 else imgT
        im, meta = _core_inputs(layout, core_angles[ci], tables)
        in_maps.append(im)
        metas.append(meta)

    nc, _ = _build_program()
    from concourse import bass_utils
    import os
    trace = bool(int(os.environ.get("CT_TRACE", "0")))
    res = bass_utils.run_bass_kernel_spmd(nc, in_maps, core_ids=list(range(NCORES)),
                                          trace=trace)
    _PROGRAM_CACHE["exec_time_ns"] = getattr(res, "exec_time_ns", None)
    _PROGRAM_CACHE["last_results"] = res

    proj = np.zeros((NTHETA, Nu), dtype=np.float64)
    done = set()
    for ci in range(NCORES):
        R = res.results[ci]["r_out"]  # [APC, NCHUNK, VP, MPAD]
        for ai, m in enumerate(metas[ci]):
            a = m["angle"]
            if a in done:
                continue
            done.add(a)
            W = m["W"]
            Mv = W + 3
            for k in range(NCHUNK):
                base = m["bxi_min"] + m["beta0"][k]
                Rk = R[ai, k].astype(np.float64)
                for v in range(VP):
                    n0 = base + v
                    if n0 >= Nu:
                        break
                    hi = min(Mv, Nu - n0)
                    proj[a, n0:n0 + hi] += Rk[v, :hi]

    if rect_angles:
        proj[rect_angles] = _host_project(img, theta[rect_angles])
    return proj.astype(np.float32)



# revision 7
# speedup vs baseline: 5.2402x; 5.2402x over previous
"""CT parallel-beam 2D forward projector on 8 Trainium2 NeuronCores.

Algorithm: variance-matched 3-tap projector. The exact trapezoid footprint is
box_|c| * box_|s| * box_1; box_|c| * box_|s| has variance (c^2+s^2)/12 = 1/12
for every angle, so replacing it with box_1 (classic linear-interp splatting)
matches mass, mean and variance exactly; the first error term is the 4th
cumulant, 2c^2s^2/120 <= 1/240 (measured 3.8e-3 rel err worst non-resonant
angle).  The two resonant angles (45, 135 deg) are computed exactly on host.

Per angle: p = a_xi*x + a_eta*e + z0, base bins bxi=floor(p_xi), beta=
floor(p_eta), g = frac_xi + frac_eta in [0,2).  Taps base+{0,1,2} get weights
U0 = relu(1-g), U1 = 1-|1-g|, U2 = relu(g-1).  CDF fields A = U0*img,
B = U2*img, C = img give slot streams: slot0=A, slot1=C-A-B, slot2=B,
realized as 5 signed one-hot matmul instances with PSUM column shifts.

Device pipeline per angle (4 eta-chunks of 128 rows, layout [eta, xi]):
  ACT : reluA_k = Relu(-fxi + (1-fe_k)), reluB_k = Relu(fxi + (fe_k-1))
  DVE : quad ops over all 4 chunks at once ([128, 4, 512] tiles): the merged
        'S-prime' xi-bin streams S_f[x] = f[x] + merge[x]*f[x+1]
        (merge[x] = [bxi[x+1]==bxi[x]])
  GPSIMD: per chunk, 2 indirect_copies (ISA limit: <=1024 dst elems each):
        [A|B] streams (1024 out) and C stream (512 out)
  PE  : 5 bf16 one-hot matmul instances, PSUM-accumulated with col shifts
        (output cols 0..511; the <=2 spill cols are added on host)
  ACT : PSUM -> SBUF drain, DMA out
Host: anti-diagonal collapse R[v,j] -> proj[n0+v+j], spill taps for angles
with W >= 511, exact path for 45/135 deg.

SPMD: one program for all 8 cores; cores 0-3 do class X angles (|cos|>=|sin|)
on img, cores 4-7 class Y on img.T; all per-angle variation is input data.
"""

import os as _os

import numpy as np

Nx = Ny = 512
Nu = 768
NTHETA = 180
HALF_U = (Nu - 1) / 2.0
NCORES = 8
APC = 23          # angles per core (89 per class over 4 cores, padded)
NCHUNK = 4        # eta chunks of 128
VP = 96           # local eta-bins per chunk (<= 91 used)
DW = 1552         # stream data width: A 0:512, z 512, B 513:1025, C 1025:1537,
                  # z 1537, pad to 1552
OW = 1536         # gather output width: A 0:512, B 512:1024, C 1024:1536
ZAB = 512         # shared zero col for A/B (within op1's data slice)
CBASE = 1025      # C stream base; op2 data slice is [1025:1538), local z 512

_PROGRAM_CACHE = {}


# --------------------------------------------------------------------------
# host tables
# --------------------------------------------------------------------------

def _angle_tables(theta_val):
    th = float(theta_val)
    c, s = np.cos(th), np.sin(th)
    cls = 0 if abs(c) >= abs(s) else 1
    a_xi, a_eta = (c, s) if cls == 0 else (s, c)
    z0 = HALF_U - 255.5 * (c + s)
    grid = np.arange(513, dtype=np.float64)
    pxi = a_xi * grid + z0
    bxi = np.floor(pxi).astype(np.int64)
    fxi = (pxi - bxi)
    peta = a_eta * np.arange(512, dtype=np.float64)
    beta = np.floor(peta).astype(np.int64)
    feta = peta - beta
    merge = (bxi[1:513] == bxi[:512]).astype(np.float64)
    merge[511] = 0.0
    return dict(cls=cls, bxi=bxi, fxi=fxi, beta=beta, feta=feta, merge=merge)


def _gather_tables(T):
    """xi-bin run starts xa[m] and the two gather index streams."""
    bxi = T["bxi"][:512]
    bxi_min = int(bxi.min())
    mloc = bxi - bxi_min
    W = int(mloc.max()) + 1
    assert W <= 512, W
    xa = np.full(W, 10 ** 9, dtype=np.int64)
    np.minimum.at(xa, mloc, np.arange(512))
    assert (xa < 512).all()
    # op1: out cols [0,1024) = A block | B block, data slice [0,1025)
    idx = np.full(OW, 0, dtype=np.int64)
    idx[0:512] = ZAB
    idx[0:W] = xa
    idx[512:1024] = ZAB
    idx[512:512 + W] = 513 + xa
    # op2: out cols [1024,1536) = C block, data slice [1025,1538) local idx
    idx[1024:1536] = 512
    idx[1024:1024 + W] = xa
    return dict(bxi_min=bxi_min, W=W, stream=idx)


def _wrap_idx(stream):
    """[OW] int -> [128, OW//16] uint16 wrapped per 16-partition groups."""
    w = stream.reshape(OW // 16, 16).T.astype(np.uint16)   # [16, OW/16]
    return np.tile(w, (8, 1))                               # [128, OW/16]


def _core_inputs(img_layout, angle_list, tables, bf16):
    """Build the input map for one core. img_layout: [512,512] f32 [eta,xi]."""
    imgP = np.zeros((512, 513), dtype=np.float32)
    imgP[:, 0:512] = img_layout
    fxm_t = np.zeros((APC, 1539), dtype=np.float32)   # fxi | merge | merge
    bias_t = np.zeros((APC, 128, 8), dtype=np.float32)
    oh_t = np.zeros((APC, 128, 2 * VP * NCHUNK), dtype=np.float32)
    idx_t = np.zeros((APC, 128, OW // 16), dtype=np.uint16)
    meta = []
    for ai, a in enumerate(angle_list):
        T = tables[a]
        G = _gather_tables(T)
        fxm_t[ai, 0:513] = T["fxi"]
        fxm_t[ai, 513:1025] = T["merge"]
        fxm_t[ai, 1026:1538] = T["merge"]
        beta = T["beta"]
        feta = T["feta"]
        beta0 = []
        for k in range(NCHUNK):
            sl = slice(k * 128, (k + 1) * 128)
            vloc = beta[sl] - beta[sl].min()
            assert vloc.min() >= 0 and vloc.max() < VP
            bias_t[ai, :, 2 * k + 0] = 1.0 - feta[sl]
            bias_t[ai, :, 2 * k + 1] = feta[sl] - 1.0
            col = 2 * VP * k
            oh_t[ai, np.arange(128), col + vloc] = 1.0
            oh_t[ai, np.arange(128), col + VP + vloc] = -1.0
            beta0.append(int(beta[sl].min()))
        idx_t[ai] = _wrap_idx(G["stream"])
        meta.append(dict(angle=a, bxi_min=G["bxi_min"], W=G["W"], beta0=beta0))
    iota_t = np.arange(96, dtype=np.float32).reshape(1, 96)
    in_map = {
        "imgL": bf16(imgP),
        "fxm_t": bf16(fxm_t),
        "bias_t": bias_t,
        "oh_t": bf16(oh_t),
        "idx_t": idx_t,
    }
    return in_map, meta


# --------------------------------------------------------------------------
# the bass program (identical for all cores)
# --------------------------------------------------------------------------

def _build_program():
    if "nc" in _PROGRAM_CACHE:
        return _PROGRAM_CACHE["nc"]

    import concourse.tile as tile
    from concourse import bacc, mybir
    from contextlib import ExitStack

    dt = mybir.dt
    AF = mybir.ActivationFunctionType
    ALU = mybir.AluOpType
    bf = dt.bfloat16

    nc = bacc.Bacc("TRN2", target_bir_lowering=False, debug=False,
                   num_devices=NCORES)

    imgL = nc.dram_tensor("imgL", [512, 513], bf, kind="ExternalInput").ap()
    fxm_t = nc.dram_tensor("fxm_t", [APC, 1539], bf, kind="ExternalInput").ap()
    bias_t = nc.dram_tensor("bias_t", [APC, 128, 8], dt.float32,
                            kind="ExternalInput").ap()
    oh_t = nc.dram_tensor("oh_t", [APC, 128, 2 * VP * NCHUNK], bf,
                          kind="ExternalInput").ap()
    idx_t = nc.dram_tensor("idx_t", [APC, 128, OW // 16], dt.uint16,
                           kind="ExternalInput").ap()
    r_out = nc.dram_tensor("r_out", [APC, NCHUNK, VP, 512], dt.float32,
                           kind="ExternalOutput").ap()

    # (field block, psum column shift, sign-column offset in oh tile)
    # oh col layout per chunk: [0:96] +onehot, [96:192] -onehot
    instances = [(0, 0, 0), (0, 1, VP), (1, 1, VP), (1, 2, 0), (2, 1, 0)]

    with tile.TileContext(nc) as tc, ExitStack() as ctx:
        img_pool = ctx.enter_context(tc.tile_pool(name="img", bufs=1))
        row_pool = ctx.enter_context(tc.tile_pool(name="rows", bufs=2))
        tab_pool = ctx.enter_context(tc.tile_pool(name="tabs", bufs=2))
        y_pool = ctx.enter_context(tc.tile_pool(name="ys", bufs=2))
        f_pool = ctx.enter_context(tc.tile_pool(name="fs", bufs=2))
        c_pool = ctx.enter_context(tc.tile_pool(name="cbuf", bufs=2))
        g_pool = ctx.enter_context(tc.tile_pool(name="gath", bufs=3))
        o_pool = ctx.enter_context(tc.tile_pool(name="outs", bufs=3))
        ps_pool = ctx.enter_context(tc.tile_pool(name="psum", bufs=3,
                                                 space="PSUM"))

        # resident image [128, 4, 513] (col 512 of each chunk zero-padded)
        img_all = img_pool.tile([128, NCHUNK, 513], bf, tag="imga")
        for k in range(NCHUNK):
            nc.sync.dma_start(img_all[:, k, :],
                              imgL[k * 128:(k + 1) * 128, :])

        for ai in range(APC):
            fxm_b = row_pool.tile([128, 3, 513], bf, tag="fxmb")
            nc.sync.dma_start(fxm_b[:, 0:2, :],
                              fxm_t[ai:ai + 1, 0:1026].to_broadcast([128, 1026]))
            nc.sync.dma_start(fxm_b[:, 2, :],
                              fxm_t[ai:ai + 1, 1026:1539].to_broadcast([128, 513]))
            fxi_b = fxm_b[:, 0, :]
            merge2 = fxm_b[:, 1:3, 0:512]

            idxt = tab_pool.tile([128, OW // 16], dt.uint16, tag="idx")
            nc.sync.dma_start(idxt[:], idx_t[ai])
            bias = tab_pool.tile([128, 8], dt.float32, tag="bias")
            nc.sync.dma_start(bias[:], bias_t[ai])
            oh = tab_pool.tile([128, 2 * VP * NCHUNK], bf, tag="oh")
            nc.sync.dma_start(oh[:], oh_t[ai])

            # ACT: per-chunk relu fields into quad tiles
            reluA = y_pool.tile([128, NCHUNK, 513], bf, tag="reluA")
            reluB = y_pool.tile([128, NCHUNK, 513], bf, tag="reluB")
            for k in range(NCHUNK):
                nc.scalar.activation(reluA[:, k, :], fxi_b, AF.Relu,
                                     bias=bias[:, 2 * k:2 * k + 1], scale=-1.0)
                nc.scalar.activation(reluB[:, k, :], fxi_b, AF.Relu,
                                     bias=bias[:, 2 * k + 1:2 * k + 2],
                                     scale=1.0)

            # DVE: quad stream builds
            imgB = f_pool.tile([128, NCHUNK, 512], bf, tag="imgB")
            for h in range(2):
                nc.vector.tensor_tensor(imgB[:, 2 * h:2 * h + 2, :],
                                        img_all[:, 2 * h:2 * h + 2, 1:513],
                                        merge2, ALU.mult)
            cb = c_pool.tile([128, NCHUNK, DW], bf, tag="cb")
            tA = f_pool.tile([128, NCHUNK, 512], bf, tag="tA")
            tA2 = f_pool.tile([128, NCHUNK, 512], bf, tag="tA2")
            nc.vector.tensor_tensor(tA[:], reluA[:, :, 0:512],
                                    img_all[:, :, 0:512], ALU.mult)
            nc.vector.tensor_tensor(tA2[:], reluA[:, :, 1:513], imgB[:],
                                    ALU.mult)
            nc.vector.tensor_tensor(cb[:, :, 0:512], tA[:], tA2[:], ALU.add)
            tB = f_pool.tile([128, NCHUNK, 512], bf, tag="tB")
            tB2 = f_pool.tile([128, NCHUNK, 512], bf, tag="tB2")
            nc.vector.tensor_tensor(tB[:], reluB[:, :, 0:512],
                                    img_all[:, :, 0:512], ALU.mult)
            nc.vector.tensor_tensor(tB2[:], reluB[:, :, 1:513], imgB[:],
                                    ALU.mult)
            nc.vector.tensor_tensor(cb[:, :, 513:1025], tB[:], tB2[:],
                                    ALU.add)
            nc.vector.tensor_tensor(cb[:, :, 1025:1537],
                                    img_all[:, :, 0:512], imgB[:], ALU.add)
            nc.vector.memset(cb[:, :, 512:513], 0.0)
            nc.vector.memset(cb[:, :, 1537:1538], 0.0)

            for k in range(NCHUNK):
                gt = g_pool.tile([128, OW], bf, tag="gt")
                nc.gpsimd.indirect_copy(gt[:, 0:1024], cb[:, k, 0:1025],
                                        idxt[:, 0:64], True)
                nc.gpsimd.indirect_copy(gt[:, 1024:1536], cb[:, k, 1025:1538],
                                        idxt[:, 64:96], True)

                ps0 = ps_pool.tile([VP, 512], dt.float32, tag="ps0")
                n_inst = len(instances)
                for pos, (f, s, oc) in enumerate(instances):
                    lhs = oh[:, 2 * VP * k + oc:2 * VP * k + oc + VP]
                    nc.tensor.matmul(ps0[:, s:512], lhs,
                                     gt[:, 512 * f:512 * f + 512 - s],
                                     start=(pos == 0), stop=(pos == n_inst - 1))
                rout = o_pool.tile([VP, 512], dt.float32, tag="rout")
                nc.scalar.copy(rout[:], ps0[:])
                nc.sync.dma_start(r_out[ai, k][:, :], rout[:])

    nc.compile()
    _PROGRAM_CACHE["nc"] = nc
    return nc


# --------------------------------------------------------------------------
# host-side exact path (resonant 45/135 deg angles) — numpy reference port
# --------------------------------------------------------------------------

def _host_project(img, theta_vals):
    y = (np.arange(Ny) - (Ny - 1) / 2.0)
    x = (np.arange(Nx) - (Nx - 1) / 2.0)
    y2d, x2d = np.meshgrid(y, x, indexing="ij")
    img_v = img.reshape(-1).astype(np.float64)
    out = np.zeros((len(theta_vals), Nu), dtype=np.float64)
    for t, th in enumerate(theta_vals):
        th = float(th)
        cos_t, sin_t = np.cos(th), np.sin(th)
        ac, asn = abs(cos_t), abs(sin_t)
        h = min(1.0 / ac if ac > 0 else np.inf, 1.0 / asn if asn > 0 else np.inf)
        b1 = abs(asn - ac)
        b2 = abs(asn + ac)
        u0 = x2d * cos_t + y2d * sin_t
        u1 = u0 - b2 / 2
        u2 = u0 - b1 / 2
        u3 = u0 + b1 / 2
        u4 = u0 + b2 / 2
        base = np.floor(u1 + HALF_U).astype(np.int64)
        den12 = (u2 - u1) + (u1 == u2)
        den34 = (u4 - u3) + (u3 == u4)
        acc = np.zeros(Nu + 8, dtype=np.float64)
        for k in range(4):
            idx = base + k
            u = idx - HALF_U
            lo, hi = u - 0.5, u + 0.5
            uA = np.maximum(u1, lo); uB = np.minimum(u2, hi)
            w = (uB > uA) * (h / (2.0 * den12)) * ((uB - u1) ** 2 - (uA - u1) ** 2)
            uA = np.maximum(u2, lo); uB = np.minimum(u3, hi)
            w = w + (uB > uA) * h * (uB - uA)
            uA = np.maximum(u3, lo); uB = np.minimum(u4, hi)
            w = w + (uB > uA) * (h / (2.0 * den34)) * ((uA - u4) ** 2 - (uB - u4) ** 2)
            np.add.at(acc, np.clip(idx.reshape(-1), 0, Nu - 1),
                      img_v * w.reshape(-1))
        out[t] = acc[:Nu]
    return out.astype(np.float32)


def _host_spill(proj, img_layout, T, a):
    """Add the taps that land at output cols >= 512 (device computes m+s <=
    511 only).  Affects angles with W >= 511; at most a few xi columns."""
    bxi = T["bxi"][:512]
    mloc = bxi - bxi.min()
    fx = T["fxi"][:512]
    fe = T["feta"]
    beta = T["beta"]
    for x in np.nonzero(mloc >= 510)[0]:
        g = fx[x] + fe                      # [512] rows
        n_base = bxi[x] + beta              # [512]
        vals = img_layout[:, x].astype(np.float64)
        for j, w in ((1, 1.0 - np.abs(1.0 - g)), (2, np.maximum(0.0, g - 1.0))):
            if mloc[x] + j < 512:
                continue
            n = n_base + j
            ok = n < Nu
            np.add.at(proj[a], n[ok], (w * vals)[ok])


# --------------------------------------------------------------------------
# main entry
# --------------------------------------------------------------------------

def kernel(img, theta):
    import ml_dtypes

    def bf16(a):
        return a.astype(ml_dtypes.bfloat16)

    img = np.asarray(img, dtype=np.float32)
    theta = np.asarray(theta, dtype=np.float32)
    assert img.shape == (Ny, Nx) and theta.shape == (NTHETA,)

    # resonant angles (the 3-tap model's only bad cases): |cos| == |sin|
    host_angles = [a for a in range(NTHETA)
                   if abs(abs(np.cos(theta[a])) - abs(np.sin(theta[a]))) < 1e-4]
    dev_angles = [a for a in range(NTHETA) if a not in host_angles]

    tables = {a: _angle_tables(theta[a]) for a in dev_angles}
    clsX = [a for a in dev_angles if tables[a]["cls"] == 0]
    clsY = [a for a in dev_angles if tables[a]["cls"] == 1]
    assert len(clsX) <= 4 * APC and len(clsY) <= 4 * APC

    def assign(lst, ncores):
        groups = [lst[i::ncores] for i in range(ncores)]
        return [g + [g[-1]] * (APC - len(g)) if g else [dev_angles[0]] * APC
                for g in groups]

    core_angles = assign(clsX, 4) + assign(clsY, 4)

    imgT = np.ascontiguousarray(img.T)
    in_maps, metas = [], []
    for ci in range(NCORES):
        layout = img if ci < 4 else imgT
        im, meta = _core_inputs(layout, core_angles[ci], tables, bf16)
        in_maps.append(im)
        metas.append(meta)

    nc = _build_program()
    from concourse import bass_utils
    trace = bool(int(_os.environ.get("CT_TRACE", "0")))
    res = bass_utils.run_bass_kernel_spmd(nc, in_maps,
                                          core_ids=list(range(NCORES)),
                                          trace=trace)
    _PROGRAM_CACHE["exec_time_ns"] = getattr(res, "exec_time_ns", None)
    _PROGRAM_CACHE["last_results"] = res

    proj = np.zeros((NTHETA, Nu), dtype=np.float64)
    done = set()
    for ci in range(NCORES):
        R = res.results[ci]["r_out"]  # [APC, NCHUNK, VP, 512]
        img_layout = img if ci < 4 else imgT
        for ai, m in enumerate(metas[ci]):
            a = m["angle"]
            if a in done:
                continue
            done.add(a)
            for k in range(NCHUNK):
                n0 = m["bxi_min"] + m["beta0"][k]
                Rk = R[ai, k].astype(np.float64)
                for v in range(VP):
                    lo = n0 + v
                    if lo >= Nu:
                        break
                    hi = min(512, Nu - lo)
                    proj[a, lo:lo + hi] += Rk[v, :hi]
            _host_spill(proj, img_layout, tables[a], a)

    if host_angles:
        proj[host_angles] = _host_project(img, theta[host_angles])
    return proj.astype(np.float32)
